# revision 16
# baseline (speedup 1.0000x reference)
"""PointWarper LBS kernel for 8 Trainium2 NeuronCores.

Strategy: the tiny MLP -> rodrigues -> kinematic-chain part is O(J) and runs
on host in float64.  The device does only the N=2M linear-blend-skinning,
reformulated as two small matmuls per point tile with a residual trick:

    xyz = pcd + Delta,   Delta = sum_j w_j * (G_j - I|0 + 0|gt) @ xyzh

Since skinning weights sum to 1, folding the identity and the global
translation into the per-bone affine residual G' makes Delta small (~5e-3),
so all device math can run in bf16 while keeping ~1e-5 absolute accuracy.

Layout (per core, 250k points, 5 points packed per column "n'"):
  wT5 [120, C]: row 24f+j = w[5n'+f, j]          (bf16, host-transposed)
  X5  [60, C]: row 12f+4a+b = xyzh[5n'+f, b]     (bf16, host-expanded)
  BD  [120, 60]: block-diag of G' [24,12], 5 copies (stationary)
  S   [60, 15]: 0/1 selection summing b           (stationary)
  mm1: A5 = BD.T @ wT5 -> [60, C] (PSUM)          A5[12f+c] = sum_j G'[j,c] w
  ACT: cast A5 -> bf16 SBUF
  DVE: Prod = A5 .* X5
  mm2: OUT = S.T @ Prod -> [15, C] (PSUM, f32)    OUT[3f+a] = Delta_a
Host adds pcd back and unpacks.
"""

import numpy as np
import ml_dtypes

import concourse.mybir as mybir
from concourse import bass, tile
from concourse.bass_utils import run_bass_kernel_spmd

BF16 = ml_dtypes.bfloat16

J = 24
N = 2_000_000
NCORES = 8
F = 5                       # points packed per column
NPC = N // NCORES           # 250_000 points per core
C = NPC // F                # 50_000 columns per core
TILE = 512
NT = (C + TILE - 1) // TILE  # 98
CP = NT * TILE              # 50_176 padded columns

TRACE = False               # test harness hook: profile the SPMD run
LAST = None                 # last BassKernelResults (for exec_time_ns)


def _relu(x):
    return np.maximum(x, 0.0)


def _chain_prod(m):
    L = m.shape[1]
    if L == 1:
        return m
    return _chain_prod(m[:, : L // 2]) @ _chain_prod(m[:, L // 2 :])


def _host_transforms(joints, t, W0, b0, W1, b1, W2, b2, W3, b3, W4,
                     parent_indices, parent_joint_ex):
    f8 = np.float64
    h = _relu(t.astype(f8) @ W0.astype(f8) + b0.astype(f8))
    h = _relu(h @ W1.astype(f8) + b1.astype(f8))
    h = _relu(h @ W2.astype(f8) + b2.astype(f8))
    h = _relu(h @ W3.astype(f8) + b3.astype(f8))
    params = (h @ W4.astype(f8)).reshape(J + 1, 4)
    gt = params[-1, :3]

    p = params[:J]
    theta = p[:, 3]
    r = p[:, :3]
    r = r / np.sqrt(1e-5 + np.sum(r * r, axis=1))[:, None]
    c, s = np.cos(theta), np.sin(theta)
    x, y, z = r[:, 0], r[:, 1], r[:, 2]
    R = np.stack([
        x * x + (1. - x * x) * c, x * y * (1. - c) - z * s, x * z * (1. - c) + y * s,
        x * y * (1. - c) + z * s, y * y + (1. - y * y) * c, y * z * (1. - c) - x * s,
        x * z * (1. - c) - y * s, y * z * (1. - c) + x * s, z * z + (1. - z * z) * c,
    ], axis=1).reshape(J, 3, 3)

    jf = joints.astype(f8)
    piv = np.concatenate([np.zeros((1, 3), f8), jf], 0)[parent_joint_ex + 1]
    trans = piv[:, :, None] + R @ (-piv[:, :, None])
    M = np.zeros((J, 4, 4), f8)
    M[:, :3, :3] = R
    M[:, :3, 3] = trans[:, :, 0]
    M[:, 3, 3] = 1.0
    Mb = np.concatenate([np.eye(4, dtype=f8)[None], M], 0)
    Mp = Mb[parent_indices + 1]          # [J, depth, 4, 4]
    bone = _chain_prod(Mp)[:, 0]         # [J, 4, 4]

    jh = np.concatenate([jf, np.ones((J, 1), f8)], 1)
    jwr = np.einsum('jab,jb->ja', bone, jh)[:, :3].astype(np.float32)

    Gres = bone[:, :3, :].copy()         # [J, 3, 4]
    Gres[:, :, 3] += gt
    Gres[:, :3, :3] -= np.eye(3, dtype=f8)
    return Gres.reshape(J, 12), jwr      # col c = 4a+b


def _build_program(repeat=1):
    nc = bass.Bass()
    wT5 = nc.declare_dram_parameter("wT5", [F * J, CP], mybir.dt.bfloat16, isOutput=False)
    X5 = nc.declare_dram_parameter("X5", [F * 12, CP], mybir.dt.bfloat16, isOutput=False)
    BD = nc.declare_dram_parameter("BD", [F * J, F * 12], mybir.dt.bfloat16, isOutput=False)
    S = nc.declare_dram_parameter("S", [F * 12, F * 3], mybir.dt.bfloat16, isOutput=False)
    OUT = nc.declare_dram_parameter("OUT", [F * 3, CP], mybir.dt.bfloat16, isOutput=True)

    with tile.TileContext(nc) as tc:
        with (
            tc.tile_pool(name="const", bufs=1) as cpool,
            tc.tile_pool(name="sb", bufs=3) as pool,
            tc.tile_pool(name="psA", bufs=3, space="PSUM") as psA,
            tc.tile_pool(name="psO", bufs=3, space="PSUM") as psO,
        ):
            bd_t = cpool.tile([F * J, F * 12], mybir.dt.bfloat16)
            nc.sync.dma_start(out=bd_t[:], in_=BD[:])
            s_t = cpool.tile([F * 12, F * 3], mybir.dt.bfloat16)
            nc.sync.dma_start(out=s_t[:], in_=S[:])

            def body():
                for ti in range(NT):
                    lo = ti * TILE
                    wt = pool.tile([F * J, TILE], mybir.dt.bfloat16)
                    nc.sync.dma_start(out=wt[:], in_=wT5[:, lo:lo + TILE])
                    xt = pool.tile([F * 12, TILE], mybir.dt.bfloat16)
                    nc.sync.dma_start(out=xt[:], in_=X5[:, lo:lo + TILE])

                    a5 = psA.tile([F * 12, TILE], mybir.dt.float32)
                    nc.tensor.matmul(out=a5[:], lhsT=bd_t[:], rhs=wt[:],
                                     start=True, stop=True)

                    pr = pool.tile([F * 12, TILE], mybir.dt.bfloat16)
                    nc.vector.tensor_mul(out=pr[:], in0=a5[:], in1=xt[:])

                    o = psO.tile([F * 3, TILE], mybir.dt.float32)
                    nc.tensor.matmul(out=o[:], lhsT=s_t[:], rhs=pr[:],
                                     start=True, stop=True)

                    ob = pool.tile([F * 3, TILE], mybir.dt.bfloat16)
                    nc.scalar.copy(out=ob[:], in_=o[:])
                    nc.sync.dma_start(out=OUT[:, lo:lo + TILE], in_=ob[:])

            if repeat == 1:
                body()
            else:
                with tc.For_i(0, repeat, 1):
                    body()

    # walrus encodes at most 1 sync wait per compute instruction; hoist
    # excess waits onto InstEventSemaphores like the Bacc pipeline does.
    mybir._bass_rust.generate_event_semaphores(nc)
    return nc


def _make_in_maps(weights, canonical_pcd, Gres):
    BDm = np.zeros((F, J, F, 12), np.float32)
    for f in range(F):
        BDm[f, :, f, :] = Gres
    BDa = BDm.reshape(F * J, F * 12).astype(BF16)

    Sm = np.zeros((F, 12, F, 3), np.float32)
    for f in range(F):
        for a in range(3):
            Sm[f, 4 * a:4 * a + 4, f, a] = 1.0
    Sa = Sm.reshape(F * 12, F * 3).astype(BF16)

    in_maps = []
    for core in range(NCORES):
        wc = weights[core * NPC:(core + 1) * NPC]           # [NPC, 24]
        wbuf = np.zeros((F * J, CP), BF16)
        wbuf[:, :C] = wc.reshape(C, F, J).transpose(1, 2, 0).reshape(F * J, C)

        pc = canonical_pcd[core * NPC:(core + 1) * NPC]     # [NPC, 3]
        xyzh = np.ones((C, F, 4), np.float32)
        xyzh[:, :, :3] = pc.reshape(C, F, 3)
        t1 = xyzh.transpose(1, 2, 0)                        # [F, 4, C]
        xbuf = np.zeros((F, 3, 4, CP), BF16)
        xbuf[:, :, :, :C] = t1[:, None, :, :]
        in_maps.append({
            "wT5": wbuf,
            "X5": xbuf.reshape(F * 12, CP),
            "BD": BDa,
            "S": Sa,
        })
    return in_maps


def kernel(weights, joints, t, canonical_pcd, W0, b0, W1, b1, W2, b2, W3, b3, W4,
           parent_indices, parent_joint_ex):
    weights = np.asarray(weights)
    canonical_pcd = np.asarray(canonical_pcd)
    parent_indices = np.asarray(parent_indices)
    parent_joint_ex = np.asarray(parent_joint_ex)

    Gres, jwr = _host_transforms(
        np.asarray(joints), np.asarray(t),
        np.asarray(W0), np.asarray(b0), np.asarray(W1), np.asarray(b1),
        np.asarray(W2), np.asarray(b2), np.asarray(W3), np.asarray(b3),
        np.asarray(W4), parent_indices, parent_joint_ex)

    in_maps = _make_in_maps(weights, canonical_pcd, Gres)

    nc = _build_program()
    try:
        br = run_bass_kernel_spmd(nc, in_maps, list(range(NCORES)), trace=TRACE)
    except ModuleNotFoundError:
        br = run_bass_kernel_spmd(nc, in_maps, list(range(NCORES)), trace=False)
    global LAST
    LAST = br
    res = br.results

    xyz = np.empty((N, 3), np.float32)
    for core in range(NCORES):
        o = np.asarray(res[core]["OUT"])[:, :C].astype(np.float32)   # [15, C]
        d = o.reshape(F, 3, C).transpose(2, 0, 1).reshape(NPC, 3)
        pc = canonical_pcd[core * NPC:(core + 1) * NPC]
        xyz[core * NPC:(core + 1) * NPC] = pc + d
    return xyz, jwr


# revision 24
# speedup vs baseline: 3.2151x; 3.2151x over previous
"""PointWarper LBS kernel for 8 Trainium2 NeuronCores.

Strategy: the tiny MLP -> rodrigues -> kinematic-chain part is O(J) and runs
on host in float64.  The device does only the N=2M linear-blend-skinning,
reformulated as two small matmuls per point tile with a residual trick:

    xyz = pcd + Delta,   Delta = sum_j w_j * (G_j - I|0 + 0|gt) @ xyzh

Since skinning weights sum to 1, folding the identity and the global
translation into the per-bone affine residual G' makes Delta small (~5e-3),
so all device math can run in bf16 while keeping ~1e-5 absolute accuracy.

Paired layout (per core, 250k points, F=5 points packed per column, tiles of
512 columns processed in PAIRS so vector/scalar ops span 128 partitions):
  wT5 [120, CP]: row 24f+j = w[5n'+f, j]            (bf16)
  X5b [128, NPAIR*512]: pair-interleaved xyzh rows   (bf16, 8 zero pad rows)
  BD64 [120, 64]: block-diag of G' [24,12] x5, 4 zero cols (stationary)
  S2  [128, 32]: 0/1 selection for both pair halves, 2 zero cols
  per pair (tiles t0,t1):
    mm1 a5[0:64]   = BD64.T @ wt[t0]   (PSUM f32, PE col base 0)
    mm1 a5[64:128] = BD64.T @ wt[t1]   (same bank, PE col base 64)
    ACT  ac = bf16(a5)        [128,512] PSUM->SBUF
    DVE  pr = ac .* xt        [128,512] all-SBUF bf16 (4x perf mode)
    mm2  og[32p:32p+32] = S2.T @ pr    (4 pairs batched per PSUM bank)
  per group of 4 pairs: ACT ob = bf16(og); DMA out [128,512].
Host adds pcd back and unpacks.
"""

import numpy as np
import ml_dtypes

import concourse.mybir as mybir
from concourse import bass, tile
from concourse.bass_utils import run_bass_kernel_spmd

BF16 = ml_dtypes.bfloat16

J = 24
N = 2_000_000
NCORES = 8
F = 5                       # points packed per column
NPC = N // NCORES           # 250_000 points per core
C = NPC // F                # 50_000 columns per core
TILE = 512
NT = 98                     # tiles per core (98*512 = 50176 >= C)
CP = NT * TILE              # 50_176 padded columns
NPAIR = NT // 2             # 49 tile pairs
XCP = NPAIR * TILE          # 25_088 columns of X5b
PPG = 3                     # pairs per output group (PSUM part. base 0/32/64)
GROUPS = (NPAIR + PPG - 1) // PPG   # 17 output groups, last ragged
OCOLS = GROUPS * TILE       # 8_704 output columns

TRACE = False               # test harness hook: profile the SPMD run
LAST = None                 # last BassKernelResults (for exec_time_ns)


def _relu(x):
    return np.maximum(x, 0.0)


def _chain_prod(m):
    L = m.shape[1]
    if L == 1:
        return m
    return _chain_prod(m[:, : L // 2]) @ _chain_prod(m[:, L // 2 :])


def _host_transforms(joints, t, W0, b0, W1, b1, W2, b2, W3, b3, W4,
                     parent_indices, parent_joint_ex):
    f8 = np.float64
    h = _relu(t.astype(f8) @ W0.astype(f8) + b0.astype(f8))
    h = _relu(h @ W1.astype(f8) + b1.astype(f8))
    h = _relu(h @ W2.astype(f8) + b2.astype(f8))
    h = _relu(h @ W3.astype(f8) + b3.astype(f8))
    params = (h @ W4.astype(f8)).reshape(J + 1, 4)
    gt = params[-1, :3]

    p = params[:J]
    theta = p[:, 3]
    r = p[:, :3]
    r = r / np.sqrt(1e-5 + np.sum(r * r, axis=1))[:, None]
    c, s = np.cos(theta), np.sin(theta)
    x, y, z = r[:, 0], r[:, 1], r[:, 2]
    R = np.stack([
        x * x + (1. - x * x) * c, x * y * (1. - c) - z * s, x * z * (1. - c) + y * s,
        x * y * (1. - c) + z * s, y * y + (1. - y * y) * c, y * z * (1. - c) - x * s,
        x * z * (1. - c) - y * s, y * z * (1. - c) + x * s, z * z + (1. - z * z) * c,
    ], axis=1).reshape(J, 3, 3)

    jf = joints.astype(f8)
    piv = np.concatenate([np.zeros((1, 3), f8), jf], 0)[parent_joint_ex + 1]
    trans = piv[:, :, None] + R @ (-piv[:, :, None])
    M = np.zeros((J, 4, 4), f8)
    M[:, :3, :3] = R
    M[:, :3, 3] = trans[:, :, 0]
    M[:, 3, 3] = 1.0
    Mb = np.concatenate([np.eye(4, dtype=f8)[None], M], 0)
    Mp = Mb[parent_indices + 1]          # [J, depth, 4, 4]
    bone = _chain_prod(Mp)[:, 0]         # [J, 4, 4]

    jh = np.concatenate([jf, np.ones((J, 1), f8)], 1)
    jwr = np.einsum('jab,jb->ja', bone, jh)[:, :3].astype(np.float32)

    Gres = bone[:, :3, :].copy()         # [J, 3, 4]
    Gres[:, :, 3] += gt
    Gres[:, :3, :3] -= np.eye(3, dtype=f8)
    return Gres.reshape(J, 12), jwr      # col c = 4a+b


def _build_program(repeat=1):
    nc = bass.Bass()
    wT5 = nc.declare_dram_parameter("wT5", [F * J, CP], mybir.dt.bfloat16, isOutput=False)
    X5b = nc.declare_dram_parameter("X5b", [128, XCP], mybir.dt.bfloat16, isOutput=False)
    BD = nc.declare_dram_parameter("BD64", [F * J, 64], mybir.dt.bfloat16, isOutput=False)
    S2 = nc.declare_dram_parameter("S2", [128, 32], mybir.dt.bfloat16, isOutput=False)
    OUT = nc.declare_dram_parameter("OUT2", [32 * PPG, OCOLS], mybir.dt.bfloat16, isOutput=True)

    with tile.TileContext(nc) as tc:
        with (
            tc.tile_pool(name="const", bufs=1) as cpool,
            tc.tile_pool(name="w", bufs=2) as wpool,
            tc.tile_pool(name="x", bufs=2) as xpool,
            tc.tile_pool(name="sb", bufs=3) as pool,
            tc.tile_pool(name="psA", bufs=3, space="PSUM") as psA,
            tc.tile_pool(name="psO", bufs=2, space="PSUM") as psO,
        ):
            bd_t = cpool.tile([F * J, 64], mybir.dt.bfloat16)
            nc.sync.dma_start(out=bd_t[:], in_=BD[:])
            s2_t = cpool.tile([128, 32], mybir.dt.bfloat16)
            nc.sync.dma_start(out=s2_t[:], in_=S2[:])

            def body():
                for g in range(GROUPS):
                    npg = min(PPG, NPAIR - PPG * g)      # pairs in this group
                    wt = wpool.tile([F * J, PPG * 1024], mybir.dt.bfloat16)
                    nc.sync.dma_start(
                        out=wt[:, : npg * 1024],
                        in_=wT5[:, g * PPG * 1024 : g * PPG * 1024 + npg * 1024])
                    xt = xpool.tile([128, PPG * TILE], mybir.dt.bfloat16)
                    nc.sync.dma_start(
                        out=xt[:, : npg * TILE],
                        in_=X5b[:, g * PPG * TILE : (g * PPG + npg) * TILE])

                    og = psO.tile([128, TILE], mybir.dt.float32)
                    for p in range(npg):
                        a5 = psA.tile([128, TILE], mybir.dt.float32)
                        nc.tensor.matmul(
                            out=a5[0:64, :], lhsT=bd_t[:],
                            rhs=wt[:, p * 1024 : p * 1024 + TILE],
                            start=True, stop=True)
                        nc.tensor.matmul(
                            out=a5[64:128, :], lhsT=bd_t[:],
                            rhs=wt[:, p * 1024 + TILE : p * 1024 + 1024],
                            start=True, stop=True)
                        ac = pool.tile([128, TILE], mybir.dt.bfloat16)
                        nc.scalar.copy(out=ac[:], in_=a5[:])
                        pr = pool.tile([128, TILE], mybir.dt.bfloat16)
                        nc.vector.tensor_mul(
                            out=pr[:], in0=ac[:],
                            in1=xt[:, p * TILE : (p + 1) * TILE])
                        nc.tensor.matmul(
                            out=og[32 * p : 32 * p + 32, :], lhsT=s2_t[:],
                            rhs=pr[:], start=True, stop=True)

                    orows = 32 * npg
                    ob = pool.tile([128, TILE], mybir.dt.bfloat16)
                    nc.scalar.copy(out=ob[:orows, :], in_=og[:orows, :])
                    nc.scalar.dma_start(
                        out=OUT[:orows, g * TILE : (g + 1) * TILE],
                        in_=ob[:orows, :])

            if repeat == 1:
                body()
            else:
                with tc.For_i(0, repeat, 1):
                    body()

    # walrus encodes at most 1 sync wait per compute instruction; hoist
    # excess waits onto InstEventSemaphores like the Bacc pipeline does.
    mybir._bass_rust.generate_event_semaphores(nc)
    return nc


def _make_in_maps(weights, canonical_pcd, Gres):
    BDm = np.zeros((F * J, 64), np.float32)
    for f in range(F):
        BDm[24 * f : 24 * f + 24, 12 * f : 12 * f + 12] = Gres
    BDa = BDm.astype(BF16)

    S2m = np.zeros((128, 32), np.float32)
    for f in range(F):
        for a in range(3):
            S2m[12 * f + 4 * a : 12 * f + 4 * a + 4, 3 * f + a] = 1.0
            S2m[64 + 12 * f + 4 * a : 64 + 12 * f + 4 * a + 4, 15 + 3 * f + a] = 1.0
    S2a = S2m.astype(BF16)

    in_maps = []
    for core in range(NCORES):
        wc = weights[core * NPC:(core + 1) * NPC]           # [NPC, 24]
        wbuf = np.zeros((F * J, CP), BF16)
        wbuf[:, :C] = wc.reshape(C, F, J).transpose(1, 2, 0).reshape(F * J, C)

        pc = canonical_pcd[core * NPC:(core + 1) * NPC]     # [NPC, 3]
        xyzh = np.ones((C, F, 4), np.float32)
        xyzh[:, :, :3] = pc.reshape(C, F, 3)
        t1 = xyzh.transpose(1, 2, 0)                        # [F, 4, C]
        x5 = np.zeros((F, 3, 4, CP), np.float32)
        x5[:, :, :, :C] = t1[:, None, :, :]
        x5 = x5.reshape(60, NPAIR, 2, TILE)                 # rows 12f+4a+b
        xb = np.zeros((128, NPAIR, TILE), np.float32)
        xb[0:60] = x5[:, :, 0]
        xb[64:124] = x5[:, :, 1]
        in_maps.append({
            "wT5": wbuf,
            "X5b": xb.reshape(128, XCP).astype(BF16),
            "BD64": BDa,
            "S2": S2a,
        })
    return in_maps


def kernel(weights, joints, t, canonical_pcd, W0, b0, W1, b1, W2, b2, W3, b3, W4,
           parent_indices, parent_joint_ex):
    weights = np.asarray(weights)
    canonical_pcd = np.asarray(canonical_pcd)
    parent_indices = np.asarray(parent_indices)
    parent_joint_ex = np.asarray(parent_joint_ex)

    Gres, jwr = _host_transforms(
        np.asarray(joints), np.asarray(t),
        np.asarray(W0), np.asarray(b0), np.asarray(W1), np.asarray(b1),
        np.asarray(W2), np.asarray(b2), np.asarray(W3), np.asarray(b3),
        np.asarray(W4), parent_indices, parent_joint_ex)

    in_maps = _make_in_maps(weights, canonical_pcd, Gres)

    nc = _build_program()
    try:
        br = run_bass_kernel_spmd(nc, in_maps, list(range(NCORES)), trace=TRACE)
    except ModuleNotFoundError:
        br = run_bass_kernel_spmd(nc, in_maps, list(range(NCORES)), trace=False)
    global LAST
    LAST = br
    res = br.results

    xyz = np.empty((N, 3), np.float32)
    for core in range(NCORES):
        o = np.asarray(res[core]["OUT2"]).astype(np.float32)   # [96, OCOLS]
        ob = o.reshape(32 * PPG, GROUPS, TILE)
        D = np.empty((15, NPAIR, 2, TILE), np.float32)
        for q in range(NPAIR):
            g, p = divmod(q, PPG)
            D[:, q, 0] = ob[32 * p : 32 * p + 15, g]
            D[:, q, 1] = ob[32 * p + 15 : 32 * p + 30, g]
        d = D.reshape(15, CP)[:, :C]                           # rows 3f+a
        d = d.reshape(F, 3, C).transpose(2, 0, 1).reshape(NPC, 3)
        pc = canonical_pcd[core * NPC:(core + 1) * NPC]
        xyz[core * NPC:(core + 1) * NPC] = pc + d
    return xyz, jwr


# revision 29
# speedup vs baseline: 4.2826x; 1.3320x over previous
"""PointWarper LBS kernel for 8 Trainium2 NeuronCores.

Strategy: the tiny MLP -> rodrigues -> kinematic-chain part is O(J) and runs
on host in float64.  The device does only the N=2M linear-blend-skinning,
reformulated as two small matmuls per point tile with a residual trick:

    xyz = pcd + Delta,   Delta = sum_j w_j * (G_j - I|0 + 0|gt) @ xyzh

Since skinning weights sum to 1, folding the identity and the global
translation into the per-bone affine residual G' makes Delta small (~5e-3),
so all device math can run in bf16 while keeping ~1e-5 absolute accuracy.

Paired layout (per core, 250k points, F=5 points packed per column, tiles of
512 columns processed in PAIRS so vector/scalar ops span 128 partitions):
  wT5 [120, CP]: row 24f+j = w[5n'+f, j]            (bf16)
  X5b [128, NPAIR*512]: pair-interleaved xyzh rows   (bf16, 8 zero pad rows)
  BD64 [120, 64]: block-diag of G' [24,12] x5, 4 zero cols (stationary)
  S2  [128, 32]: 0/1 selection for both pair halves, 2 zero cols
  per pair (tiles t0,t1):
    mm1 a5[0:64]   = BD64.T @ wt[t0]   (PSUM f32, PE col base 0)
    mm1 a5[64:128] = BD64.T @ wt[t1]   (same bank, PE col base 64)
    ACT  ac = bf16(a5)        [128,512] PSUM->SBUF
    DVE  pr = ac .* xt        [128,512] all-SBUF bf16 (4x perf mode)
    mm2  og[32p:32p+32] = S2.T @ pr    (4 pairs batched per PSUM bank)
  per group of 4 pairs: ACT ob = bf16(og); DMA out [128,512].
Host adds pcd back and unpacks.
"""

import numpy as np
import ml_dtypes

import concourse.mybir as mybir
from concourse import bass, tile
from concourse.bass_utils import run_bass_kernel_spmd

BF16 = ml_dtypes.bfloat16

J = 24
N = 2_000_000
NCORES = 8
F = 5                       # points packed per column
NPC = N // NCORES           # 250_000 points per core
C = NPC // F                # 50_000 columns per core
TILE = 512
NT = 98                     # tiles per core (98*512 = 50176 >= C)
CP = NT * TILE              # 50_176 padded columns
NPAIR = NT // 2             # 49 tile pairs
XCP = NPAIR * TILE          # 25_088 columns of X5b
PPG = 3                     # pairs per output group (PSUM part. base 0/32/64)
GROUPS = (NPAIR + PPG - 1) // PPG   # 17 output groups, last ragged
OCOLS = GROUPS * TILE       # 8_704 output columns

TRACE = False               # test harness hook: profile the SPMD run
LAST = None                 # last BassKernelResults (for exec_time_ns)


def _relu(x):
    return np.maximum(x, 0.0)


def _chain_prod(m):
    L = m.shape[1]
    if L == 1:
        return m
    return _chain_prod(m[:, : L // 2]) @ _chain_prod(m[:, L // 2 :])


def _host_transforms(joints, t, W0, b0, W1, b1, W2, b2, W3, b3, W4,
                     parent_indices, parent_joint_ex):
    f8 = np.float64
    h = _relu(t.astype(f8) @ W0.astype(f8) + b0.astype(f8))
    h = _relu(h @ W1.astype(f8) + b1.astype(f8))
    h = _relu(h @ W2.astype(f8) + b2.astype(f8))
    h = _relu(h @ W3.astype(f8) + b3.astype(f8))
    params = (h @ W4.astype(f8)).reshape(J + 1, 4)
    gt = params[-1, :3]

    p = params[:J]
    theta = p[:, 3]
    r = p[:, :3]
    r = r / np.sqrt(1e-5 + np.sum(r * r, axis=1))[:, None]
    c, s = np.cos(theta), np.sin(theta)
    x, y, z = r[:, 0], r[:, 1], r[:, 2]
    R = np.stack([
        x * x + (1. - x * x) * c, x * y * (1. - c) - z * s, x * z * (1. - c) + y * s,
        x * y * (1. - c) + z * s, y * y + (1. - y * y) * c, y * z * (1. - c) - x * s,
        x * z * (1. - c) - y * s, y * z * (1. - c) + x * s, z * z + (1. - z * z) * c,
    ], axis=1).reshape(J, 3, 3)

    jf = joints.astype(f8)
    piv = np.concatenate([np.zeros((1, 3), f8), jf], 0)[parent_joint_ex + 1]
    trans = piv[:, :, None] + R @ (-piv[:, :, None])
    M = np.zeros((J, 4, 4), f8)
    M[:, :3, :3] = R
    M[:, :3, 3] = trans[:, :, 0]
    M[:, 3, 3] = 1.0
    Mb = np.concatenate([np.eye(4, dtype=f8)[None], M], 0)
    Mp = Mb[parent_indices + 1]          # [J, depth, 4, 4]
    bone = _chain_prod(Mp)[:, 0]         # [J, 4, 4]

    jh = np.concatenate([jf, np.ones((J, 1), f8)], 1)
    jwr = np.einsum('jab,jb->ja', bone, jh)[:, :3].astype(np.float32)

    Gres = bone[:, :3, :].copy()         # [J, 3, 4]
    Gres[:, :, 3] += gt
    Gres[:, :3, :3] -= np.eye(3, dtype=f8)
    return Gres.reshape(J, 12), jwr      # col c = 4a+b


def _build_program(repeat=1):
    nc = bass.Bass()
    wT5 = nc.declare_dram_parameter("wT5", [F * J, CP], mybir.dt.float8e4, isOutput=False)
    X5b = nc.declare_dram_parameter("X5b", [128, XCP], mybir.dt.bfloat16, isOutput=False)
    BD = nc.declare_dram_parameter("BD64", [F * J, 64], mybir.dt.bfloat16, isOutput=False)
    S2 = nc.declare_dram_parameter("S2", [128, 32], mybir.dt.bfloat16, isOutput=False)
    OUT = nc.declare_dram_parameter("OUT2", [32 * PPG, OCOLS], mybir.dt.bfloat16, isOutput=True)

    with tile.TileContext(nc) as tc:
        with (
            tc.tile_pool(name="const", bufs=1) as cpool,
            tc.tile_pool(name="w", bufs=2) as wpool,
            tc.tile_pool(name="x", bufs=2) as xpool,
            tc.tile_pool(name="sb", bufs=3) as pool,
            tc.tile_pool(name="ob", bufs=2) as obpool,
            tc.tile_pool(name="psA", bufs=3, space="PSUM") as psA,
            tc.tile_pool(name="psO", bufs=2, space="PSUM") as psO,
        ):
            bd_t = cpool.tile([F * J, 64], mybir.dt.bfloat16)
            nc.sync.dma_start(out=bd_t[:], in_=BD[:])
            s2_t = cpool.tile([128, 32], mybir.dt.bfloat16)
            nc.sync.dma_start(out=s2_t[:], in_=S2[:])

            def body():
                pend = None                              # delayed out DMA
                for g in range(GROUPS):
                    npg = min(PPG, NPAIR - PPG * g)      # pairs in this group
                    wt = wpool.tile([F * J, PPG * 1024], mybir.dt.float8e4)
                    nc.sync.dma_start(
                        out=wt[:, : npg * 1024],
                        in_=wT5[:, g * PPG * 1024 : g * PPG * 1024 + npg * 1024])
                    xt = xpool.tile([128, PPG * TILE], mybir.dt.bfloat16)
                    nc.sync.dma_start(
                        out=xt[:, : npg * TILE],
                        in_=X5b[:, g * PPG * TILE : (g * PPG + npg) * TILE])
                    if pend is not None:
                        ob_p, orows_p, g_p = pend
                        nc.sync.dma_start(
                            out=OUT[:orows_p, g_p * TILE : (g_p + 1) * TILE],
                            in_=ob_p[:orows_p, :])
                        pend = None

                    og = psO.tile([128, TILE], mybir.dt.float32)
                    for p in range(npg):
                        a5 = psA.tile([128, TILE], mybir.dt.float32)
                        nc.tensor.matmul(
                            out=a5[0:64, :], lhsT=bd_t[:],
                            rhs=wt[:, p * 1024 : p * 1024 + TILE],
                            start=True, stop=True)
                        nc.tensor.matmul(
                            out=a5[64:128, :], lhsT=bd_t[:],
                            rhs=wt[:, p * 1024 + TILE : p * 1024 + 1024],
                            start=True, stop=True)
                        ac = pool.tile([128, TILE], mybir.dt.bfloat16)
                        nc.scalar.copy(out=ac[:], in_=a5[:])
                        pr = pool.tile([128, TILE], mybir.dt.bfloat16)
                        nc.vector.tensor_mul(
                            out=pr[:], in0=ac[:],
                            in1=xt[:, p * TILE : (p + 1) * TILE])
                        nc.tensor.matmul(
                            out=og[32 * p : 32 * p + 32, :], lhsT=s2_t[:],
                            rhs=pr[:], start=True, stop=True)

                    orows = 32 * npg
                    ob = obpool.tile([128, TILE], mybir.dt.bfloat16)
                    nc.vector.tensor_copy(out=ob[:orows, :], in_=og[:orows, :])
                    pend = (ob, orows, g)
                ob_p, orows_p, g_p = pend
                nc.sync.dma_start(
                    out=OUT[:orows_p, g_p * TILE : (g_p + 1) * TILE],
                    in_=ob_p[:orows_p, :])

            if repeat == 1:
                body()
            else:
                with tc.For_i(0, repeat, 1):
                    body()

    # walrus encodes at most 1 sync wait per compute instruction; hoist
    # excess waits onto InstEventSemaphores like the Bacc pipeline does.
    mybir._bass_rust.generate_event_semaphores(nc)
    return nc


def _make_in_maps(weights, canonical_pcd, Gres):
    BDm = np.zeros((F * J, 64), np.float32)
    for f in range(F):
        BDm[24 * f : 24 * f + 24, 12 * f : 12 * f + 12] = Gres
    BDa = BDm.astype(BF16)

    S2m = np.zeros((128, 32), np.float32)
    for f in range(F):
        for a in range(3):
            S2m[12 * f + 4 * a : 12 * f + 4 * a + 4, 3 * f + a] = 1.0
            S2m[64 + 12 * f + 4 * a : 64 + 12 * f + 4 * a + 4, 15 + 3 * f + a] = 1.0
    S2a = S2m.astype(BF16)

    in_maps = []
    for core in range(NCORES):
        wc = weights[core * NPC:(core + 1) * NPC]           # [NPC, 24]
        wbuf = np.zeros((F * J, CP), ml_dtypes.float8_e4m3)
        wbuf[:, :C] = wc.reshape(C, F, J).transpose(1, 2, 0).reshape(F * J, C).astype(
            ml_dtypes.float8_e4m3)

        pc = canonical_pcd[core * NPC:(core + 1) * NPC]     # [NPC, 3]
        xyzh = np.ones((C, F, 4), np.float32)
        xyzh[:, :, :3] = pc.reshape(C, F, 3)
        t1 = xyzh.transpose(1, 2, 0)                        # [F, 4, C]
        x5 = np.zeros((F, 3, 4, CP), np.float32)
        x5[:, :, :, :C] = t1[:, None, :, :]
        x5 = x5.reshape(60, NPAIR, 2, TILE)                 # rows 12f+4a+b
        xb = np.zeros((128, NPAIR, TILE), np.float32)
        xb[0:60] = x5[:, :, 0]
        xb[64:124] = x5[:, :, 1]
        in_maps.append({
            "wT5": wbuf,
            "X5b": xb.reshape(128, XCP).astype(BF16),
            "BD64": BDa,
            "S2": S2a,
        })
    return in_maps


def kernel(weights, joints, t, canonical_pcd, W0, b0, W1, b1, W2, b2, W3, b3, W4,
           parent_indices, parent_joint_ex):
    weights = np.asarray(weights)
    canonical_pcd = np.asarray(canonical_pcd)
    parent_indices = np.asarray(parent_indices)
    parent_joint_ex = np.asarray(parent_joint_ex)

    Gres, jwr = _host_transforms(
        np.asarray(joints), np.asarray(t),
        np.asarray(W0), np.asarray(b0), np.asarray(W1), np.asarray(b1),
        np.asarray(W2), np.asarray(b2), np.asarray(W3), np.asarray(b3),
        np.asarray(W4), parent_indices, parent_joint_ex)

    in_maps = _make_in_maps(weights, canonical_pcd, Gres)

    nc = _build_program()
    try:
        br = run_bass_kernel_spmd(nc, in_maps, list(range(NCORES)), trace=TRACE)
    except ModuleNotFoundError:
        br = run_bass_kernel_spmd(nc, in_maps, list(range(NCORES)), trace=False)
    global LAST
    LAST = br
    res = br.results

    xyz = np.empty((N, 3), np.float32)
    for core in range(NCORES):
        o = np.asarray(res[core]["OUT2"]).astype(np.float32)   # [96, OCOLS]
        ob = o.reshape(32 * PPG, GROUPS, TILE)
        D = np.empty((15, NPAIR, 2, TILE), np.float32)
        for q in range(NPAIR):
            g, p = divmod(q, PPG)
            D[:, q, 0] = ob[32 * p : 32 * p + 15, g]
            D[:, q, 1] = ob[32 * p + 15 : 32 * p + 30, g]
        d = D.reshape(15, CP)[:, :C]                           # rows 3f+a
        d = d.reshape(F, 3, C).transpose(2, 0, 1).reshape(NPC, 3)
        pc = canonical_pcd[core * NPC:(core + 1) * NPC]
        xyz[core * NPC:(core + 1) * NPC] = pc + d
    return xyz, jwr
